# revision 20
# baseline (speedup 1.0000x reference)
"""Conv2d(128->256, 3x3, pad=1) + sync-BatchNorm(train) + ReLU on 8 TRN2 cores.

Strategy (data-parallel, hardcoded for x:[32,128,56,56] w:[256,128,3,3]):
  - Shard batch 32 -> 4 images/core across 8 cores.
  - Host pre-pads x to 58x58, casts x/w to bf16, pre-transposes weights to
    [Cin, o_tile, tap, o] so each tap's [128,128] weight tile is contiguous
    per partition -> walrus uses Fast Weight Load (LDWEIGHTS overlaps MATMUL).
  - Conv = implicit GEMM: Cin=128 is the partition/contraction dim; each 3x3
    tap is one bf16 matmul ([128,128] weights x [128,448] shifted-image view)
    accumulated in fp32 PSUM. Output rows in 7 groups of 8 rows (8*56=448 <=
    512 fp32 PSUM bank); chunks of 4+3 groups, tap-major inside a chunk.
    Dummy warmup matmuls ramp the PE clock during the input DMA wait.
  - BN train-mode: conv bias cancels exactly ((y+b) - mean(y+b) == y - mean).
    sum(y) folds into the PSUM-evacuating ACT Copy (accum_out, fp32
    accumulator); sum(y^2) is a DVE scalar_tensor_tensor over the bf16 SBUF
    copy, so PSUM is released by the Copy alone.
  - Sync-BN: one small AllReduce per 128-channel otile. An AR on this fabric
    costs ~12-45us trigger->usable, heavily inflated by concurrent DMA-ring
    traffic, so the schedule keeps rings quiet inside every AR window:
      * o=1 convs run FIRST; AR(1) triggers at mid-kernel and completes under
        o=0's convs. o=1 then normalizes+stores mid-phase (scalar relus for
        n0/n1 between evac copies, gpsimd tensor_scalar pairs for n2/n3),
        with stores finishing before the o=0 conv tail.
      * AR(0) triggers right after the last evac into quiet rings; only
        o=0's 6.4MB of stores remain after it.
    A dummy warmup AR at kernel start absorbs cross-core skew under the
    input DMA phase.
  - Stats finalize (global sums -> scale/shift) runs entirely on DVE using a
    fused (var+eps)^-0.5 tensor_scalar pow op - no scalar-engine Sqrt, so an
    AR wait can never stall the evac-copy queue that gates PSUM reuse.
"""

import os

import numpy as np
import ml_dtypes

import concourse.bass as bass
import concourse.mybir as mybir
import concourse.tile as tile
from concourse import bacc

F32 = mybir.dt.float32
BF16 = mybir.dt.bfloat16

N_CORES = 8
IMGS = 4            # images per core
CIN = 128
COUT = 256
H = W = 56
HP = WP = 58        # padded
NG = 7              # row-groups per image (8 rows each)
RG = 8              # rows per group
GROUP = RG * W      # 448
BANK = 512          # fp32 elems per PSUM bank
EPS = 1e-5
COUNT = float(32 * H * W)   # global BN element count per channel
N_WARM_MM = 14      # dummy matmuls to ramp the PE clock (~3us)

AF = mybir.ActivationFunctionType
ALU = mybir.AluOpType

CHUNKS = [(0, 4), (4, 3)]   # (first group, n groups) -> 4+3 PSUM banks


def build_nc() -> bass.Bass:
    no_ar = bool(os.environ.get("CONVACT_NO_AR"))
    # Bacc (not raw Bass): its compile pipeline legalizes semaphore waits
    # (TRN2 allows at most one wait per instruction; matmul waits move to
    # ldweights / event-semaphore instructions).
    nc = bacc.Bacc()
    xp_d = nc.declare_dram_parameter("xp", [IMGS, CIN, HP, WP], BF16, isOutput=False)
    wt_d = nc.declare_dram_parameter("wt", [CIN, 2, 9, 128], BF16, isOutput=False)
    gb_d = nc.declare_dram_parameter("gb", [128, 4], F32, isOutput=False)
    out_d = nc.declare_dram_parameter("out", [IMGS, COUT, H, W], F32, isOutput=True)

    with tile.TileContext(nc) as tc:
        with (
            tc.tile_pool(name="const", bufs=1) as cpool,
            tc.tile_pool(name="psum", bufs=2, space="PSUM") as ppool,
            tc.tile_pool(name="scrp", bufs=2) as spool,
            tc.tile_pool(name="stgp", bufs=4) as tpool,
            tc.tile_pool(name="dram", bufs=1, space="DRAM") as dpool,
        ):
            Wt = cpool.tile([128, 2, 9, 128], BF16)
            GB = cpool.tile([128, 4], F32)
            X = cpool.tile([128, IMGS, HP, WP], BF16)
            Y = cpool.tile([128, 2, IMGS, NG, GROUP], BF16)
            Ssum = cpool.tile([128, 2, IMGS * 2], F32)
            Ssq = cpool.tile([128, 2, IMGS * 2], F32)
            ST = cpool.tile([128, 2, 2], F32)    # packed (sum, sumsq) per otile
            G = cpool.tile([128, 2, 2], F32)     # post-AR global (sum, sumsq)
            mean = cpool.tile([128, 2], F32)
            e2 = cpool.tile([128, 2], F32)
            msq = cpool.tile([128, 2], F32)
            var = cpool.tile([128, 2], F32)
            std = cpool.tile([128, 2], F32)
            inv = cpool.tile([128, 2], F32)
            sc = cpool.tile([128, 2], F32)
            sh = cpool.tile([128, 2], F32)
            epsT = cpool.tile([128, 1], F32)
            warm = cpool.tile([128, 1], F32)
            warmX = cpool.tile([128, GROUP], BF16)
            bnc_in = [
                dpool.tile([128, 2], F32, name=f"bnc_in{i}") for i in range(2)
            ]
            bnc_out = [
                dpool.tile([128, 2], F32, name=f"bnc_out{i}") for i in range(2)
            ]
            warm_in = dpool.tile([128, 1], F32)
            warm_out = dpool.tile([128, 1], F32)

            # ---- warmup collective: absorbs cross-core skew + warms the CC
            # path, hidden under the input DMAs / convs. Its 512B payload DMA
            # goes out before the bulk input transfers. ----
            nc.vector.memset(warm[:, :], 0.0)
            nc.gpsimd.dma_start(warm_in[:, :], warm[:, :])
            if not no_ar:
                nc.gpsimd.collective_compute(
                    "AllReduce",
                    ALU.add,
                    replica_groups=[list(range(N_CORES))],
                    ins=[warm_in.opt()],
                    outs=[warm_out.opt()],
                )
            nc.vector.memset(warmX[:, :], 0.0)
            nc.vector.memset(epsT[:, :], EPS)

            # ---- loads: critical-path first. X0 split across two rings so
            # both halves land before the first two chunks need them. ----
            nc.sync.dma_start(X[:, 0, 0:34, :], xp_d[0, :, 0:34, :])
            nc.gpsimd.dma_start(X[:, 0, 34:HP, :], xp_d[0, :, 34:HP, :])
            nc.scalar.dma_start(Wt[:, 1], wt_d[:, 1])
            nc.scalar.dma_start(GB[:, :], gb_d[:, :])
            nc.scalar.dma_start(Wt[:, 0], wt_d[:, 0])
            nc.sync.dma_start(X[:, 1], xp_d[1])
            nc.gpsimd.dma_start(X[:, 2], xp_d[2])
            nc.scalar.dma_start(X[:, 3], xp_d[3])

            # ---- PE clock warmup on dummy data while X lands ----
            wps = ppool.tile([128, 4, BANK], F32, tag="ps", name="wps")
            for i in range(N_WARM_MM):
                nc.tensor.matmul(
                    wps[:, i % 4, 0:GROUP],
                    warmX[:, 0:128],
                    warmX[:, :],
                    start=True,
                    stop=True,
                    skip_group_check=True,
                )

            def conv_chunk(o, n, ci):
                g0, ngr = CHUNKS[ci]
                ps = ppool.tile([128, 4, BANK], F32, tag="ps")
                # tap-major: consecutive matmuls share the same weight tile
                for t in range(9):
                    kh, kw = divmod(t, 3)
                    for gg in range(ngr):
                        g = g0 + gg
                        rhs = X[:, n, g * RG + kh : g * RG + kh + RG, kw : kw + W]
                        nc.tensor.matmul(
                            ps[:, gg, 0:GROUP],
                            Wt[:, o, t, :],
                            rhs,
                            start=(t == 0),
                            stop=(t == 8),
                        )
                col = n * 2 + ci
                ysl = Y[:, o, n, g0 : g0 + ngr, :]
                # evacuate PSUM -> Y (bf16) and fold sum(y) into the same ACT
                # op (fp32 accumulator); PSUM is released by this Copy alone.
                nc.scalar.activation(
                    ysl,
                    ps[:, 0:ngr, 0:GROUP],
                    AF.Copy,
                    accum_out=Ssum[:, o, col : col + 1],
                )
                # sum(y^2) on DVE from the SBUF copy (bf16 in/out, fp32 accum)
                scr = spool.tile([128, 4, GROUP], BF16, tag="scr")
                nc.vector.scalar_tensor_tensor(
                    scr[:, 0:ngr, :],
                    ysl,
                    1.0,
                    ysl,
                    ALU.mult,
                    ALU.mult,
                    accum_out=Ssq[:, o, col : col + 1],
                )

            def stats_trigger(o, ring=None):
                # pack local (sum, sumsq) and kick off the per-otile AllReduce
                nc.vector.reduce_sum(
                    ST[:, o, 0:1], Ssum[:, o : o + 1, :], axis=mybir.AxisListType.X
                )
                nc.vector.reduce_sum(
                    ST[:, o, 1:2], Ssq[:, o : o + 1, :], axis=mybir.AxisListType.X
                )
                (ring or nc.gpsimd).dma_start(
                    bnc_in[o][:, :], ST[:, o, :], single_packet=True
                )
                if no_ar:
                    nc.gpsimd.dma_start(bnc_out[o][:, :], bnc_in[o][:, :])
                else:
                    nc.gpsimd.collective_compute(
                        "AllReduce",
                        ALU.add,
                        replica_groups=[list(range(N_CORES))],
                        ins=[bnc_in[o].opt()],
                        outs=[bnc_out[o].opt()],
                    )

            def stats_finalize(o):
                # global stats -> per-channel scale/shift for this otile
                nc.gpsimd.dma_start(G[:, o, :], bnc_out[o][:, :], single_packet=True)
                inv_cnt = (N_CORES if no_ar else 1.0) / COUNT
                osl = slice(o, o + 1)
                nc.vector.tensor_scalar_mul(mean[:, osl], G[:, o, 0:1], inv_cnt)
                nc.vector.tensor_scalar_mul(e2[:, osl], G[:, o, 1:2], inv_cnt)
                nc.vector.tensor_mul(msq[:, osl], mean[:, osl], mean[:, osl])
                nc.vector.tensor_sub(var[:, osl], e2[:, osl], msq[:, osl])
                nc.scalar.activation(
                    std[:, osl], var[:, osl], AF.Sqrt, bias=epsT[:, 0:1]
                )
                nc.vector.reciprocal(inv[:, osl], std[:, osl])
                nc.vector.tensor_mul(sc[:, osl], GB[:, o : o + 1], inv[:, osl])
                nc.vector.tensor_mul(msq[:, osl], mean[:, osl], sc[:, osl])
                nc.vector.tensor_sub(sh[:, osl], GB[:, 2 + o : 3 + o], msq[:, osl])

            def store(o, n, ci, stage, ring):
                g0, ngr = CHUNKS[ci]
                ring.dma_start(
                    out_d[
                        n, o * 128 : (o + 1) * 128, g0 * RG : (g0 + ngr) * RG, :
                    ].rearrange("p h w -> p (h w)"),
                    stage[:, 0:ngr, :].rearrange("p a b -> p (a b)"),
                )

            def norm_store_act(o, n, ci, ring):
                # scalar-engine normalize: one fused Relu(y*sc+sh) pass
                g0, ngr = CHUNKS[ci]
                stage = tpool.tile([128, 4, GROUP], F32, tag="stage")
                nc.scalar.activation(
                    stage[:, 0:ngr, :],
                    Y[:, o, n, g0 : g0 + ngr, :],
                    AF.Relu,
                    bias=sh[:, o : o + 1],
                    scale=sc[:, o : o + 1],
                )
                store(o, n, ci, stage, ring)

            def norm_store_ts(o, n, ci, ring, eng):
                # DVE/GpSimd normalize: tensor_scalar mult-add then max(.,0)
                g0, ngr = CHUNKS[ci]
                stage = tpool.tile([128, 4, GROUP], F32, tag="stage")
                eng.tensor_scalar(
                    stage[:, 0:ngr, :],
                    Y[:, o, n, g0 : g0 + ngr, :],
                    sc[:, o : o + 1],
                    sh[:, o : o + 1],
                    ALU.mult,
                    ALU.add,
                )
                eng.tensor_scalar_max(stage[:, 0:ngr, :], stage[:, 0:ngr, :], 0.0)
                store(o, n, ci, stage, ring)

            # ---- o=1 convs first, then AR(1) trigger at mid-kernel ----
            for n in range(IMGS):
                conv_chunk(1, n, 0)
                conv_chunk(1, n, 1)
            stats_trigger(1)

            # ---- o=0 convs n0..n2; AR(1) completes underneath ----
            for n in range(3):
                conv_chunk(0, n, 0)
                conv_chunk(0, n, 1)

            # ---- o=1 normalize+store mid-phase on scalar: the sc/sh deps
            # gate them on AR(1), which lands around when the scalar queue
            # reaches here. The relu block is interleaved with o=0 n3's
            # chunks so n3's evac copies aren't queued behind 8 relus (the
            # 2-buffer PSUM rotation keeps the matmul stream safe either
            # way). Stores finish before the conv tail so AR(0)'s window
            # stays quiet. ----
            stats_finalize(1)
            norm_store_act(1, 0, 0, nc.sync)
            norm_store_act(1, 0, 1, nc.gpsimd)
            norm_store_act(1, 1, 0, nc.sync)
            conv_chunk(0, 3, 0)
            norm_store_act(1, 1, 1, nc.gpsimd)
            norm_store_act(1, 2, 0, nc.sync)
            conv_chunk(0, 3, 1)
            norm_store_act(1, 2, 1, nc.gpsimd)
            norm_store_act(1, 3, 0, nc.sync)
            norm_store_act(1, 3, 1, nc.gpsimd)
            # AR(0) payload rides the scalar ring, which is idle here
            stats_trigger(0, ring=nc.scalar)

            # ---- o=0 normalize+store tail, split scalar/DVE + 3 rings ----
            stats_finalize(0)
            rings = [nc.sync, nc.gpsimd, nc.scalar]
            for i, n in enumerate(range(IMGS)):
                norm_store_act(0, n, 0, rings[(2 * i) % 3])
                norm_store_ts(0, n, 1, rings[(2 * i + 1) % 3], nc.vector)
    return nc


_CACHE: dict = {}


def _get_nc() -> bass.Bass:
    if "nc" not in _CACHE:
        nc = build_nc()
        # Bacc.finalize runs the compile pipeline (wait legalization, register
        # allocation, nop fusion) - required before handing BIR to walrus.
        nc.finalize()
        _CACHE["nc"] = nc
    return _CACHE["nc"]


def _prep_inputs(x, weight, gamma, beta):
    x = np.ascontiguousarray(np.asarray(x, dtype=np.float32))
    w = np.asarray(weight, dtype=np.float32)
    gamma = np.asarray(gamma, dtype=np.float32)
    beta = np.asarray(beta, dtype=np.float32)

    B = x.shape[0]
    per = B // N_CORES
    xp = np.zeros((B, CIN, HP, WP), ml_dtypes.bfloat16)
    xp[:, :, 1 : 1 + H, 1 : 1 + W] = x.astype(ml_dtypes.bfloat16)
    # [Cout,Cin,3,3] -> [Cin, otile, tap, o]: tap-contiguous weight tiles
    wt = np.ascontiguousarray(
        w.transpose(1, 0, 2, 3)
        .reshape(CIN, 2, 128, 9)
        .transpose(0, 1, 3, 2)
        .astype(ml_dtypes.bfloat16)
    )
    gb = np.ascontiguousarray(
        np.stack([gamma[:128], gamma[128:], beta[:128], beta[128:]], axis=1)
    )
    return [
        {"xp": xp[c * per : (c + 1) * per], "wt": wt, "gb": gb}
        for c in range(N_CORES)
    ]


def run(x, weight, bias=None, gamma=None, beta=None, trace=False, **kw):
    """Full-input entry; returns (out, BassKernelResults)."""
    from concourse.bass_utils import run_bass_kernel_spmd

    in_maps = _prep_inputs(x, weight, gamma, beta)
    res = run_bass_kernel_spmd(
        _get_nc(), in_maps, list(range(N_CORES)), trace=trace, **kw
    )
    out = np.concatenate([res.results[c]["out"] for c in range(N_CORES)], axis=0)
    return out, res


def kernel(x, weight, bias=None, gamma=None, beta=None):
    out, _ = run(x, weight, bias=bias, gamma=gamma, beta=beta, trace=False)
    return out


# revision 25
# speedup vs baseline: 1.0077x; 1.0077x over previous
"""Conv2d(128->256, 3x3, pad=1) + sync-BatchNorm(train) + ReLU on 8 TRN2 cores.

Strategy (data-parallel, hardcoded for x:[32,128,56,56] w:[256,128,3,3]):
  - Shard batch 32 -> 4 images/core across 8 cores.
  - Host pre-pads x to 58x58, casts x/w to bf16, pre-transposes weights to
    [Cin, o_tile, tap, o] so each tap's [128,128] weight tile is contiguous
    per partition -> walrus uses Fast Weight Load (LDWEIGHTS overlaps MATMUL).
  - Conv = implicit GEMM: Cin=128 is the partition/contraction dim; each 3x3
    tap is one bf16 matmul ([128,128] weights x [128,448] shifted-image view)
    accumulated in fp32 PSUM. Output rows in 7 groups of 8 rows (8*56=448 <=
    512 fp32 PSUM bank); chunks of 4+3 groups, tap-major inside a chunk.
    Dummy warmup matmuls ramp the PE clock during the input DMA wait.
  - BN train-mode: conv bias cancels exactly ((y+b) - mean(y+b) == y - mean).
    sum(y) folds into the PSUM-evacuating ACT Copy (accum_out, fp32
    accumulator); sum(y^2) is a DVE scalar_tensor_tensor over the bf16 SBUF
    copy, so PSUM is released by the Copy alone.
  - Sync-BN: one small AllReduce per 128-channel otile. An AR on this fabric
    costs ~12-45us trigger->usable, heavily inflated by concurrent DMA-ring
    traffic, so the schedule keeps rings quiet inside every AR window:
      * o=1 convs run FIRST; AR(1) triggers at mid-kernel and completes under
        o=0's convs. o=1 then normalizes+stores mid-phase (scalar relus for
        n0/n1 between evac copies, gpsimd tensor_scalar pairs for n2/n3),
        with stores finishing before the o=0 conv tail.
      * AR(0) triggers right after the last evac into quiet rings; only
        o=0's 6.4MB of stores remain after it.
    A dummy warmup AR at kernel start absorbs cross-core skew under the
    input DMA phase.
  - Stats finalize (global sums -> scale/shift) runs entirely on DVE using a
    fused (var+eps)^-0.5 tensor_scalar pow op - no scalar-engine Sqrt, so an
    AR wait can never stall the evac-copy queue that gates PSUM reuse.
"""

import os

import numpy as np
import ml_dtypes

import concourse.bass as bass
import concourse.mybir as mybir
import concourse.tile as tile
from concourse import bacc

F32 = mybir.dt.float32
BF16 = mybir.dt.bfloat16

N_CORES = 8
IMGS = 4            # images per core
CIN = 128
COUT = 256
H = W = 56
HP = WP = 58        # padded
NG = 7              # row-groups per image (8 rows each)
RG = 8              # rows per group
GROUP = RG * W      # 448
BANK = 512          # fp32 elems per PSUM bank
EPS = 1e-5
COUNT = float(32 * H * W)   # global BN element count per channel
N_WARM_MM = 14      # dummy matmuls to ramp the PE clock (~3us)

AF = mybir.ActivationFunctionType
ALU = mybir.AluOpType

CHUNKS = [(0, 4), (4, 3)]   # (first group, n groups) -> 4+3 PSUM banks


def build_nc() -> bass.Bass:
    no_ar = bool(os.environ.get("CONVACT_NO_AR"))
    # Bacc (not raw Bass): its compile pipeline legalizes semaphore waits
    # (TRN2 allows at most one wait per instruction; matmul waits move to
    # ldweights / event-semaphore instructions).
    nc = bacc.Bacc()
    xp_d = nc.declare_dram_parameter("xp", [IMGS, CIN, HP, WP], BF16, isOutput=False)
    wt_d = nc.declare_dram_parameter("wt", [CIN, 2, 9, 128], BF16, isOutput=False)
    gb_d = nc.declare_dram_parameter("gb", [128, 4], F32, isOutput=False)
    out_d = nc.declare_dram_parameter("out", [IMGS, COUT, H, W], F32, isOutput=True)

    with tile.TileContext(nc) as tc:
        with (
            tc.tile_pool(name="const", bufs=1) as cpool,
            tc.tile_pool(name="psum", bufs=2, space="PSUM") as ppool,
            tc.tile_pool(name="scrp", bufs=2) as spool,
            tc.tile_pool(name="stgp", bufs=4) as tpool,
            tc.tile_pool(name="dram", bufs=1, space="DRAM") as dpool,
        ):
            Wt = cpool.tile([128, 2, 9, 128], BF16)
            GB = cpool.tile([128, 4], F32)
            X = cpool.tile([128, IMGS, HP, WP], BF16)
            Y = cpool.tile([128, 2, IMGS, NG, GROUP], BF16)
            Ssum = cpool.tile([128, 2, IMGS * 2], F32)
            Ssq = cpool.tile([128, 2, IMGS * 2], F32)
            ST = cpool.tile([128, 2, 2], F32)    # packed (sum, sumsq) per otile
            G = cpool.tile([128, 2, 2], F32)     # post-AR global (sum, sumsq)
            mean = cpool.tile([128, 2], F32)
            e2 = cpool.tile([128, 2], F32)
            msq = cpool.tile([128, 2], F32)
            var = cpool.tile([128, 2], F32)
            std = cpool.tile([128, 2], F32)
            inv = cpool.tile([128, 2], F32)
            sc = cpool.tile([128, 2], F32)
            sh = cpool.tile([128, 2], F32)
            epsT = cpool.tile([128, 1], F32)
            warm = cpool.tile([128, 1], F32)
            warmX = cpool.tile([128, GROUP], BF16)
            bnc_in = [
                dpool.tile([128, 2], F32, name=f"bnc_in{i}") for i in range(2)
            ]
            bnc_out = [
                dpool.tile([128, 2], F32, name=f"bnc_out{i}") for i in range(2)
            ]
            warm_in = dpool.tile([128, 1], F32)
            warm_out = dpool.tile([128, 1], F32)

            # ---- warmup collective: absorbs cross-core skew + warms the CC
            # path, hidden under the input DMAs / convs. Its 512B payload DMA
            # goes out before the bulk input transfers. ----
            nc.vector.memset(warm[:, :], 0.0)
            nc.gpsimd.dma_start(warm_in[:, :], warm[:, :])
            if not no_ar:
                nc.gpsimd.collective_compute(
                    "AllReduce",
                    ALU.add,
                    replica_groups=[list(range(N_CORES))],
                    ins=[warm_in.opt()],
                    outs=[warm_out.opt()],
                )
            nc.vector.memset(warmX[:, :], 0.0)
            nc.vector.memset(epsT[:, :], EPS)

            # ---- loads: critical-path first. X0 split across two rings so
            # both halves land before the first two chunks need them. ----
            nc.sync.dma_start(X[:, 0, 0:34, :], xp_d[0, :, 0:34, :])
            nc.gpsimd.dma_start(X[:, 0, 34:HP, :], xp_d[0, :, 34:HP, :])
            nc.scalar.dma_start(Wt[:, 1], wt_d[:, 1])
            nc.scalar.dma_start(GB[:, :], gb_d[:, :])
            nc.scalar.dma_start(Wt[:, 0], wt_d[:, 0])
            nc.sync.dma_start(X[:, 1], xp_d[1])
            nc.gpsimd.dma_start(X[:, 2], xp_d[2])
            nc.scalar.dma_start(X[:, 3], xp_d[3])

            # ---- PE clock warmup on dummy data while X lands ----
            wps = ppool.tile([128, 4, BANK], F32, tag="ps", name="wps")
            for i in range(N_WARM_MM):
                nc.tensor.matmul(
                    wps[:, i % 4, 0:GROUP],
                    warmX[:, 0:128],
                    warmX[:, :],
                    start=True,
                    stop=True,
                    skip_group_check=True,
                )

            def conv_chunk(o, n, ci):
                g0, ngr = CHUNKS[ci]
                ps = ppool.tile([128, 4, BANK], F32, tag="ps")
                # tap-major: consecutive matmuls share the same weight tile
                for t in range(9):
                    kh, kw = divmod(t, 3)
                    for gg in range(ngr):
                        g = g0 + gg
                        rhs = X[:, n, g * RG + kh : g * RG + kh + RG, kw : kw + W]
                        nc.tensor.matmul(
                            ps[:, gg, 0:GROUP],
                            Wt[:, o, t, :],
                            rhs,
                            start=(t == 0),
                            stop=(t == 8),
                        )
                col = n * 2 + ci
                ysl = Y[:, o, n, g0 : g0 + ngr, :]
                # evacuate PSUM -> Y (bf16) on DVE, folding sum(y) into the
                # same op (fp32 accumulator); PSUM is released by this op
                # alone, and the DVE queue carries no AR-dependent work so
                # PSUM turnaround can never stall on a collective.
                nc.vector.tensor_scalar(
                    ysl,
                    ps[:, 0:ngr, 0:GROUP],
                    1.0,
                    0.0,
                    ALU.mult,
                    ALU.add,
                    accum_out=Ssum[:, o, col : col + 1],
                )
                # sum(y^2) also on DVE from the bf16 SBUF copy — the scalar
                # queue carries ONLY normalize work, so its AR-gated relus
                # can start the moment stats land.
                scr = spool.tile([128, 4, GROUP], BF16, tag="scr")
                nc.vector.scalar_tensor_tensor(
                    scr[:, 0:ngr, :],
                    ysl,
                    1.0,
                    ysl,
                    ALU.mult,
                    ALU.mult,
                    accum_out=Ssq[:, o, col : col + 1],
                )

            def stats_trigger(o, ring=None):
                # pack local (sum, sumsq) and kick off the per-otile AllReduce
                nc.vector.reduce_sum(
                    ST[:, o, 0:1], Ssum[:, o : o + 1, :], axis=mybir.AxisListType.X
                )
                nc.vector.reduce_sum(
                    ST[:, o, 1:2], Ssq[:, o : o + 1, :], axis=mybir.AxisListType.X
                )
                (ring or nc.gpsimd).dma_start(
                    bnc_in[o][:, :], ST[:, o, :], single_packet=True
                )
                if no_ar:
                    nc.gpsimd.dma_start(bnc_out[o][:, :], bnc_in[o][:, :])
                else:
                    nc.gpsimd.collective_compute(
                        "AllReduce",
                        ALU.add,
                        replica_groups=[list(range(N_CORES))],
                        ins=[bnc_in[o].opt()],
                        outs=[bnc_out[o].opt()],
                    )

            def stats_finalize(o):
                # global stats -> per-channel scale/shift for this otile
                nc.gpsimd.dma_start(G[:, o, :], bnc_out[o][:, :], single_packet=True)
                inv_cnt = (N_CORES if no_ar else 1.0) / COUNT
                osl = slice(o, o + 1)
                nc.vector.tensor_scalar_mul(mean[:, osl], G[:, o, 0:1], inv_cnt)
                nc.vector.tensor_scalar_mul(e2[:, osl], G[:, o, 1:2], inv_cnt)
                nc.vector.tensor_mul(msq[:, osl], mean[:, osl], mean[:, osl])
                nc.vector.tensor_sub(var[:, osl], e2[:, osl], msq[:, osl])
                nc.scalar.activation(
                    std[:, osl], var[:, osl], AF.Sqrt, bias=epsT[:, 0:1]
                )
                nc.vector.reciprocal(inv[:, osl], std[:, osl])
                nc.vector.tensor_mul(sc[:, osl], GB[:, o : o + 1], inv[:, osl])
                nc.vector.tensor_mul(msq[:, osl], mean[:, osl], sc[:, osl])
                nc.vector.tensor_sub(sh[:, osl], GB[:, 2 + o : 3 + o], msq[:, osl])

            def store(o, n, ci, stage, ring):
                g0, ngr = CHUNKS[ci]
                ring.dma_start(
                    out_d[
                        n, o * 128 : (o + 1) * 128, g0 * RG : (g0 + ngr) * RG, :
                    ].rearrange("p h w -> p (h w)"),
                    stage[:, 0:ngr, :].rearrange("p a b -> p (a b)"),
                )

            def norm_store_act(o, n, ci, ring):
                # scalar-engine normalize: one fused Relu(y*sc+sh) pass
                g0, ngr = CHUNKS[ci]
                stage = tpool.tile([128, 4, GROUP], F32, tag="stage")
                nc.scalar.activation(
                    stage[:, 0:ngr, :],
                    Y[:, o, n, g0 : g0 + ngr, :],
                    AF.Relu,
                    bias=sh[:, o : o + 1],
                    scale=sc[:, o : o + 1],
                )
                store(o, n, ci, stage, ring)

            def norm_store_ts(o, n, ci, ring, eng):
                # DVE/GpSimd normalize: tensor_scalar mult-add then max(.,0)
                g0, ngr = CHUNKS[ci]
                stage = tpool.tile([128, 4, GROUP], F32, tag="stage")
                eng.tensor_scalar(
                    stage[:, 0:ngr, :],
                    Y[:, o, n, g0 : g0 + ngr, :],
                    sc[:, o : o + 1],
                    sh[:, o : o + 1],
                    ALU.mult,
                    ALU.add,
                )
                eng.tensor_scalar_max(stage[:, 0:ngr, :], stage[:, 0:ngr, :], 0.0)
                store(o, n, ci, stage, ring)

            # ---- o=1 convs first, then AR(1) trigger at mid-kernel ----
            for n in range(IMGS):
                conv_chunk(1, n, 0)
                conv_chunk(1, n, 1)
            stats_trigger(1)

            # ---- o=0 convs n0..n2; AR(1) completes underneath ----
            for n in range(3):
                conv_chunk(0, n, 0)
                conv_chunk(0, n, 1)

            # finalize(1)'s DVE ops sit between o=0 n2's and n3's evacs —
            # the DVE queue reaches them right around when AR(1)'s result
            # lands, so the evac stream doesn't stall.
            stats_finalize(1)

            # ---- o=1 normalize+store mid-phase: the scalar queue holds
            # nothing but Sqrt+relus, so these start the moment AR(1)'s
            # scale/shift are ready and their stores drain before AR(0)'s
            # mesh window opens. ----
            for n in range(IMGS):
                norm_store_act(1, n, 0, nc.sync)
                norm_store_act(1, n, 1, nc.gpsimd)

            conv_chunk(0, 3, 0)
            conv_chunk(0, 3, 1)
            # AR(0) payload rides the sync queue/ring (mid stores done)
            stats_trigger(0, ring=nc.sync)

            # ---- o=0 normalize+store tail, split scalar/DVE + 3 rings ----
            stats_finalize(0)
            rings = [nc.sync, nc.gpsimd, nc.scalar]
            for i, n in enumerate(range(IMGS)):
                norm_store_act(0, n, 0, rings[(2 * i) % 3])
                norm_store_ts(0, n, 1, rings[(2 * i + 1) % 3], nc.vector)
    return nc


_CACHE: dict = {}


def _get_nc() -> bass.Bass:
    if "nc" not in _CACHE:
        nc = build_nc()
        # Bacc.finalize runs the compile pipeline (wait legalization, register
        # allocation, nop fusion) - required before handing BIR to walrus.
        nc.finalize()
        _CACHE["nc"] = nc
    return _CACHE["nc"]


def _prep_inputs(x, weight, gamma, beta):
    x = np.ascontiguousarray(np.asarray(x, dtype=np.float32))
    w = np.asarray(weight, dtype=np.float32)
    gamma = np.asarray(gamma, dtype=np.float32)
    beta = np.asarray(beta, dtype=np.float32)

    B = x.shape[0]
    per = B // N_CORES
    xp = np.zeros((B, CIN, HP, WP), ml_dtypes.bfloat16)
    xp[:, :, 1 : 1 + H, 1 : 1 + W] = x.astype(ml_dtypes.bfloat16)
    # [Cout,Cin,3,3] -> [Cin, otile, tap, o]: tap-contiguous weight tiles
    wt = np.ascontiguousarray(
        w.transpose(1, 0, 2, 3)
        .reshape(CIN, 2, 128, 9)
        .transpose(0, 1, 3, 2)
        .astype(ml_dtypes.bfloat16)
    )
    gb = np.ascontiguousarray(
        np.stack([gamma[:128], gamma[128:], beta[:128], beta[128:]], axis=1)
    )
    return [
        {"xp": xp[c * per : (c + 1) * per], "wt": wt, "gb": gb}
        for c in range(N_CORES)
    ]


def run(x, weight, bias=None, gamma=None, beta=None, trace=False, **kw):
    """Full-input entry; returns (out, BassKernelResults)."""
    from concourse.bass_utils import run_bass_kernel_spmd

    in_maps = _prep_inputs(x, weight, gamma, beta)
    res = run_bass_kernel_spmd(
        _get_nc(), in_maps, list(range(N_CORES)), trace=trace, **kw
    )
    out = np.concatenate([res.results[c]["out"] for c in range(N_CORES)], axis=0)
    return out, res


def kernel(x, weight, bias=None, gamma=None, beta=None):
    out, _ = run(x, weight, bias=bias, gamma=gamma, beta=beta, trace=False)
    return out
